# revision 48
# baseline (speedup 1.0000x reference)
"""Trainium2 Bass kernel for nn_AttentionModel (pointer-network decode step).

Data-parallel over 8 NeuronCores: batch 512 -> 64 samples/core; weights
replicated.  Per core the kernel streams the per-sample K/V slabs from HBM
once (~75 MB in bf16/fp8) and computes:

  self-attn over (K_sa | k_sa) -> LN -> enc attention (masked) -> LN ->
  MLP -> LN -> single-head tanh-clipped pointer scores -> softmax weights.

Key structure:
  - K streams (bf16, host-transposed d-major) and V streams (fp8e4m3,
    keys-major) feed per-head score / AV matmuls; softmax, LN, residuals
    and the final output stay fp32.  Mixed bf16-weight x fp8-value
    matmuls are used for AV.
  - ALL stream-load DMA triggers are emitted up front on the sync/gpsimd
    queues (alternating per sample; a small critical prefix of weights
    rides ahead of the floods); ring-slot semaphores pace them, so
    prefetch flows straight through layer transitions instead of stalling
    behind compute program order, and no compute engine ever blocks on a
    starved DMA ring.
  - per-sample 16-head scores live in PSUM as 32-row bands at 32-aligned
    offsets (4 samples per [128, nkeys] tile via PE column-group packing).
    PE transposes of the bf16 weight tile, a block-diagonal head mask and
    a block-diagonal ones-matmul fold per-head value products back to
    [4, 512] batch-major rows.
  - the appended self-attn token's score column is computed once for all
    64 samples (qm-masked q*k reduction) and injected via a DRAM-bounce
    rearrange; its value contribution is a rank-1 elementwise update of
    attn1 (no PE work).
"""

import numpy as np
import ml_dtypes
from contextlib import ExitStack

import concourse.bass as bass
import concourse.tile as tile
from concourse import bacc, mybir
from concourse.bass_utils import run_bass_kernel_spmd

f32 = mybir.dt.float32
bf16 = mybir.dt.bfloat16
fp8 = mybir.dt.float8e4
AF = mybir.ActivationFunctionType
ALU = mybir.AluOpType
AX = mybir.AxisListType

P = 128          # SBUF partitions
NCORES = 8
B = 512          # full batch
BC = B // NCORES # batch per core (64)
D = 512          # model dim
H = 16           # heads
DH = 32          # head dim
NK = 251         # encoder keys (nb_nodes + 1)
NP = 256         # encoder keys padded to 256
T = 256          # self-attn cache length (new token appended on device)
NG = BC // 4     # sample groups of 4 (one [128, n] psum tile each)

WNAMES = ["Wq_sa", "Wk_sa", "Wv_sa", "W0_sa", "Wq_a", "W0_a", "W1", "W2", "Wqf"]
# weight matmuls whose bias is applied on batch-major [64, 512] rows
BM_BIAS = {"Wv_sa", "W0_sa", "W0_a", "W2"}

_cache = {}


# ----------------------------------------------------------------------------
# program builder
# ----------------------------------------------------------------------------

def _build_program(flags):
    """flags = (use_bias tuple aligned with WNAMES, ln_affine tuple of 3)."""
    use_bias = dict(zip(WNAMES, flags[0]))
    ln_affine = flags[1]

    nc = bacc.Bacc("TRN2", target_bir_lowering=False, debug=False)

    def din(name, shape, dt=f32):
        return nc.dram_tensor(name, shape, dt, kind="ExternalInput").ap()

    hT_d = din("hT", [D, BC], bf16)
    hbm_d = din("h_bm", [BC, D])
    ksaT_d = din("ksaT", [BC, P, 4, T], bf16)
    vsa_d = din("vsa", [BC, P, 2, D], fp8)
    ka0T_d = din("ka0T", [BC, P, 4, NP], bf16)
    kafT_d = din("kafT", [BC, P, 4, NP], bf16)
    va0_d = din("va0", [BC, P, 2, D], fp8)
    maskc_d = din("maskc", [BC, NP], bf16)   # compact -1e9/0 mask rows
    W_d = {n: din("W_" + n, [D, D], bf16) for n in WNAMES}
    b_d = {n: din("b_" + n, [P, 4]) for n in WNAMES if use_bias[n]}
    bf_d = {n: din("bf_" + n, [1, D]) for n in WNAMES
            if use_bias[n] and n in BM_BIAS}
    if any(ln_affine):
        lnp_d = din("lnp", [6, D])
    ident_d = din("ident", [P, P], bf16)
    ones4_d = din("ones4", [P, 4], bf16)
    bm4_d = din("bm4", [P, D])
    qm_d = din("qm", [P, 4 * DH], bf16)

    out_d = nc.dram_tensor("out", [BC, NK], f32, kind="ExternalOutput").ap()

    def bcast_row(ap2d, i, n):
        row = ap2d[i:i + 1, :]
        return bass.AP(tensor=row.tensor, offset=row.offset,
                       ap=[[0, n]] + list(row.ap)[1:])

    def mask_bcast(g):
        """[128, NP] tile AP reading maskc rows 4g..4g+4, each row x32."""
        m = maskc_d[4 * g:4 * g + 4, :]
        return bass.AP(tensor=m.tensor, offset=m.offset,
                       ap=[[NP, 4], [0, 32], [1, NP]])

    with tile.TileContext(nc) as tc, ExitStack() as ctx:
        consts = ctx.enter_context(tc.tile_pool(name="consts", bufs=1))
        acts = ctx.enter_context(tc.tile_pool(name="acts", bufs=1))
        small = ctx.enter_context(tc.tile_pool(name="small", bufs=16))
        big_tmp = ctx.enter_context(tc.tile_pool(name="big_tmp", bufs=2))
        dscr = ctx.enter_context(tc.tile_pool(name="dscr", bufs=1, space="DRAM"))
        # streaming pools: one ring each across l1/l2/final
        pk = ctx.enter_context(tc.tile_pool(name="kstream", bufs=46))
        pv = ctx.enter_context(tc.tile_pool(name="vstream", bufs=12))
        pm = ctx.enter_context(tc.tile_pool(name="mp", bufs=16))
        pqbd = ctx.enter_context(tc.tile_pool(name="qbdp", bufs=3))
        pw = ctx.enter_context(tc.tile_pool(name="escp", bufs=3))
        pwt = ctx.enter_context(tc.tile_pool(name="wtp", bufs=3))
        pex = ctx.enter_context(tc.tile_pool(name="exp", bufs=3))
        pa4 = ctx.enter_context(tc.tile_pool(name="a4p", bufs=3))
        # attention PSUM pools (12KB/partition held; 4KB left for proj/residual)
        psc = ctx.enter_context(tc.tile_pool(name="pscp", bufs=2, space="PSUM"))
        pspt = ctx.enter_context(tc.tile_pool(name="psptp", bufs=2, space="PSUM"))
        psmall = ctx.enter_context(tc.tile_pool(name="psmallp", bufs=1, space="PSUM"))

        # ------------------------------------------------------------------
        # constants / weights (sync queue: needed-first weights + h; scalar
        # queue: the rest)
        # ------------------------------------------------------------------
        # all pre-compute loads go on the sync queue (a pure trigger engine)
        # BEFORE the stream floods, so no compute engine ever stalls on a
        # starved DMA ring.
        # critical startup prefix on the sync queue: only what the first
        # attention groups need; everything needed later prefixes the gpsimd
        # queue so neither delays the other or stalls a compute engine.
        x0T = acts.tile([P, 4, BC], bf16, name="x0T", tag="x0T")
        nc.sync.dma_start(out=x0T, in_=hT_d.rearrange("(c p) b -> p c b", p=P))
        Wt, bt, bft = {}, {}, {}
        EARLY_W = ("Wq_sa", "Wk_sa", "Wv_sa")
        for n in WNAMES:
            Wt[n] = consts.tile([P, 4, D], bf16, name="W_" + n, tag="W_" + n)
            if n in EARLY_W:
                nc.sync.dma_start(out=Wt[n],
                                  in_=W_d[n].rearrange("(c p) m -> p c m", p=P))
        qm = consts.tile([P, 4 * DH], bf16, name="qm", tag="qm")
        nc.sync.dma_start(out=qm, in_=qm_d)
        ident = consts.tile([P, P], bf16, name="ident", tag="ident")
        nc.sync.dma_start(out=ident, in_=ident_d)
        ones4 = consts.tile([P, 4], bf16, name="ones4", tag="ones4")
        nc.sync.dma_start(out=ones4, in_=ones4_d)
        bm4 = consts.tile([P, D], f32, name="bm4", tag="bm4")
        nc.sync.dma_start(out=bm4, in_=bm4_d)
        eps = consts.tile([P, 1], f32, name="eps", tag="eps")
        nc.vector.memset(eps, 1e-5)
        # later-needed loads: gpsimd-queue prefix
        for n in WNAMES:
            if n not in EARLY_W:
                nc.gpsimd.dma_start(out=Wt[n],
                                    in_=W_d[n].rearrange("(c p) m -> p c m", p=P))
        h_bm = acts.tile([BC, D], f32, name="h_bm", tag="h_bm")
        nc.gpsimd.dma_start(out=h_bm, in_=hbm_d)
        for n in WNAMES:
            if use_bias[n]:
                bt[n] = consts.tile([P, 4], f32, name="b_" + n, tag="b_" + n)
                nc.gpsimd.dma_start(out=bt[n], in_=b_d[n])
                if n in BM_BIAS:
                    bft[n] = consts.tile([BC, D], f32, name="bf_" + n, tag="bf_" + n)
                    nc.gpsimd.dma_start(out=bft[n], in_=bcast_row(bf_d[n], 0, BC))

        lng, lnb = [None] * 3, [None] * 3
        for i in range(3):
            if ln_affine[i]:
                lng[i] = consts.tile([BC, D], f32, name=f"lng{i}", tag=f"lng{i}")
                nc.gpsimd.dma_start(out=lng[i], in_=bcast_row(lnp_d, 2 * i, BC))
                lnb[i] = consts.tile([BC, D], f32, name=f"lnb{i}", tag=f"lnb{i}")
                nc.gpsimd.dma_start(out=lnb[i], in_=bcast_row(lnp_d, 2 * i + 1, BC))

        # ------------------------------------------------------------------
        # ALL stream loads, emitted up front (ring semaphores pace them),
        # alternating queues per sample for balanced bytes
        # ------------------------------------------------------------------
        l1_kt, l1_vt, l2_kt, l2_vt, fin_kt = [], [], [], [], []
        l2_mt, fin_mt = [], []
        for b in range(BC):
            ek = nc.sync if b % 2 == 0 else nc.gpsimd
            ev = nc.gpsimd if b % 2 == 0 else nc.sync
            kt = pk.tile([P, 4, T], bf16, name="kt4", tag="kt4")
            ek.dma_start(out=kt, in_=ksaT_d[b])
            l1_kt.append(kt)
            vt = pv.tile([P, 2, D], fp8, name="vt2", tag="vt2")
            ev.dma_start(out=vt, in_=vsa_d[b])
            l1_vt.append(vt)
        for b in range(BC):
            ek = nc.sync if b % 2 == 0 else nc.gpsimd
            ev = nc.gpsimd if b % 2 == 0 else nc.sync
            kt = pk.tile([P, 4, NP], bf16, name="kt4", tag="kt4")
            ek.dma_start(out=kt, in_=ka0T_d[b])
            l2_kt.append(kt)
            vt = pv.tile([P, 2, D], fp8, name="vt2", tag="vt2")
            ev.dma_start(out=vt, in_=va0_d[b])
            l2_vt.append(vt)
        for b in range(BC):
            kt = pk.tile([P, 4, NP], bf16, name="kt4", tag="kt4")
            eng = nc.sync if b % 2 == 0 else nc.gpsimd
            eng.dma_start(out=kt, in_=kafT_d[b])
            fin_kt.append(kt)
        for g in range(NG):
            mt = pm.tile([P, NP], bf16, name="mt2", tag="mt2")
            nc.gpsimd.dma_start(out=mt, in_=mask_bcast(g))
            l2_mt.append(mt)
        fin_mt = l2_mt

        # ------------------------------------------------------------------
        # helpers
        # ------------------------------------------------------------------
        def proj_dmajor(dst, wname, src_T, relu=False):
            """dst[:, mc, :] (d-major bf16 [128, 4, 64]) = act(x @ W + b)."""
            with tc.tile_pool(name="pp_" + wname, bufs=2, space="PSUM") as pp:
                for mc in range(4):
                    ps = pp.tile([P, BC], f32, name="ps", tag="ps")
                    for kc in range(4):
                        nc.tensor.matmul(
                            ps,
                            lhsT=Wt[wname][:, kc, mc * P:(mc + 1) * P],
                            rhs=src_T[:, kc, :],
                            start=(kc == 0), stop=(kc == 3),
                        )
                    bias = bt[wname][:, mc:mc + 1] if use_bias[wname] else 0.0
                    func = AF.Relu if relu else AF.Identity
                    nc.scalar.activation(dst[:, mc, :], ps, func, bias=bias, scale=1.0)

        def mm_batchmajor(ps, src_T, wname):
            """ps [64, 512] = x @ W   (lhsT = x^T chunks, W as moving)."""
            for kc in range(4):
                nc.tensor.matmul(
                    ps,
                    lhsT=src_T[:, kc, :],
                    rhs=Wt[wname][:, kc, :],
                    start=(kc == 0), stop=(kc == 3),
                )

        def transpose_bm(dst_T, src_bm):
            """[64, 512] bf16 batch-major -> d-major bf16 [128, 4, 64]."""
            with tc.tile_pool(name="ptr", bufs=2, space="PSUM") as pt:
                for c in range(4):
                    ps = pt.tile([P, BC], bf16, name="ps", tag="ps")
                    nc.tensor.transpose(ps, src_bm[:, c * P:(c + 1) * P],
                                        ident[0:BC, 0:BC])
                    nc.scalar.copy(dst_T[:, c, :], ps)

        def layer_norm(dst_bm, t_bm, idx):
            stats = small.tile([BC, 6], f32, name="stats", tag="stats")
            nc.vector.bn_stats(stats, t_bm)
            mv = small.tile([BC, 2], f32, name="mv", tag="mv")
            nc.vector.bn_aggr(mv, stats)
            sd = small.tile([BC, 1], f32, name="sd", tag="sd")
            nc.scalar.activation(sd, mv[:, 1:2], AF.Sqrt, bias=eps[0:BC], scale=1.0)
            rstd = small.tile([BC, 1], f32, name="rstd", tag="rstd")
            nc.vector.reciprocal(rstd, sd)
            nmr = small.tile([BC, 1], f32, name="nmr", tag="nmr")
            nc.vector.scalar_tensor_tensor(out=nmr, in0=mv[:, 0:1], scalar=-1.0,
                                           in1=rstd, op0=ALU.mult, op1=ALU.mult)
            if ln_affine[idx]:
                xn = big_tmp.tile([BC, D], f32, name="xn", tag="xn")
                nc.scalar.activation(xn, t_bm, AF.Identity, bias=nmr, scale=rstd)
                nc.vector.tensor_mul(xn, xn, lng[idx])
                nc.vector.tensor_add(dst_bm, xn, lnb[idx])
            else:
                nc.scalar.activation(dst_bm, t_bm, AF.Identity, bias=nmr, scale=rstd)

        def residual_ln(dst_bm, dst_T, src_T, wname, x_prev_bm, idx):
            """dst = LN(x_prev + src @ W + b); also d-major bf16 dst_T."""
            with tc.tile_pool(name="pr_" + wname, bufs=1, space="PSUM") as pr:
                ps = pr.tile([BC, D], f32, name="ps", tag="ps")
                mm_batchmajor(ps, src_T, wname)
                t_bm = big_tmp.tile([BC, D], f32, name="t_bm", tag="t_bm")
                nc.vector.tensor_add(t_bm, ps, x_prev_bm)
                if use_bias[wname]:
                    nc.vector.tensor_add(t_bm, t_bm, bft[wname])
                layer_norm(dst_bm, t_bm, idx)
            dst_bf = big_tmp.tile([BC, D], bf16, name="dbf", tag="dbf_" + wname)
            nc.scalar.copy(dst_bf, dst_bm)
            transpose_bm(dst_T, dst_bf)

        def qbd_group(dst, src_T, g):
            """dst [P, 16, DH] bf16: block-diag per-head queries, 4 samples.

            dst[:, 4j+c, :] = qm[:, c*DH:(c+1)*DH] * src_T[:, c, 4g+j]
            done in one broadcast vector op (free dims j, c, dh).
            """
            o = dst[:, :, :]
            a = src_T[:, :, :]
            m = qm[:, :]
            out_ap = bass.AP(tensor=o.tensor, offset=o.offset,
                             ap=[list(o.ap[0]), [4 * DH, 4], [DH, 4], [1, DH]])
            in1 = bass.AP(tensor=a.tensor, offset=a.offset + 4 * g,
                          ap=[list(a.ap[0]), [1, 4], [BC, 4], [0, DH]])
            in0 = bass.AP(tensor=m.tensor, offset=m.offset,
                          ap=[list(m.ap[0]), [0, 4], [DH, 4], [1, DH]])
            nc.vector.tensor_mul(out_ap, in0, in1)

        # ------------------------------------------------------------------
        # projections from x0 = h_t
        # ------------------------------------------------------------------
        q_saT = acts.tile([P, 4, BC], bf16, name="q_saT", tag="q_saT")
        proj_dmajor(q_saT, "Wq_sa", x0T)
        k_saT = acts.tile([P, 4, BC], bf16, name="k_saT", tag="k_saT")
        proj_dmajor(k_saT, "Wk_sa", x0T)

        v_bm = acts.tile([BC, D], bf16, name="v_bm", tag="v_bm")
        with tc.tile_pool(name="pv0", bufs=1, space="PSUM") as pv0:
            psv = pv0.tile([BC, D], f32, name="psv", tag="psv")
            mm_batchmajor(psv, x0T, "Wv_sa")
            if use_bias["Wv_sa"]:
                nc.vector.tensor_add(v_bm, psv, bft["Wv_sa"])
            else:
                nc.scalar.copy(v_bm, psv)

        # extra-token self-attn scores for all samples, batch-major:
        # sxb[b, h] = q_b . k_b per head (cols h>=16 exact 0), bounced
        # through DRAM into band layout sx4[32j+h, g].
        qk = acts.tile([P, 4, BC], bf16, name="qk", tag="qk")
        nc.vector.tensor_mul(qk, q_saT, k_saT)
        sxb = acts.tile([BC, DH], f32, name="sxb", tag="sxb")
        with tc.tile_pool(name="px", bufs=1, space="PSUM") as px:
            ps_x = px.tile([BC, DH], f32, name="ps_x", tag="ps_x")
            for c in range(4):
                nc.tensor.matmul(ps_x, lhsT=qk[:, c, :],
                                 rhs=qm[:, c * DH:(c + 1) * DH],
                                 start=(c == 0), stop=(c == 3))
            nc.scalar.copy(sxb, ps_x)
        sxd = dscr.tile([BC, DH], f32, name="sxd", tag="sxd")
        nc.scalar.dma_start(out=sxd, in_=sxb)
        sx4 = acts.tile([P, NG], f32, name="sx4", tag="sx4")
        _s = sxd[:, :]
        nc.scalar.dma_start(
            out=sx4,
            in_=bass.AP(tensor=_s.tensor, offset=_s.offset,
                        ap=[[DH, 4], [1, DH], [4 * DH, NG]]))

        # ------------------------------------------------------------------
        # layer 1: self-attention over (K_sa | k_sa)
        # ------------------------------------------------------------------
        attn1 = acts.tile([BC, D], bf16, name="attn1", tag="attn1")
        ewall = acts.tile([P, NG], f32, name="ewall", tag="ewall")
        wxp = acts.tile([BC, H], f32, name="wxp", tag="wxp")

        def attention(q_T, kts, vts, mts, dst, layer, dst_T=None):
            wcols = T + 1 if layer == 1 else NP
            tchunks = [(0, P), (1, P)]
            for g in range(NG):
                qbd = pqbd.tile([P, 16, DH], bf16, name="qbd", tag="qbd")
                qbd_group(qbd, q_T, g)
                ps_sc = psc.tile([P, wcols], f32, name="ps_sc", tag="ps_sc")
                for c in range(4):
                    for j in range(4):
                        b = 4 * g + j
                        nc.tensor.matmul(
                            ps_sc[32 * j:32 * j + 32, 0:NP],
                            lhsT=qbd[:, 4 * j + c, :],
                            rhs=kts[b][:, c, :],
                            start=(c == 0), stop=(c == 3),
                            tile_position=(0, 32 * j))
                if layer == 1:
                    nc.scalar.copy(ps_sc[:, T:T + 1], sx4[:, g:g + 1])
                else:
                    nc.vector.tensor_add(ps_sc, ps_sc, mts[g])
                negmax = small.tile([P, 1], f32, name="negmax", tag="negmax")
                nc.vector.tensor_reduce(negmax, ps_sc, axis=AX.X,
                                        op=ALU.max, negate=True)
                esc = pw.tile([P, wcols], bf16, name="esc", tag="esc")
                sumexp = small.tile([P, 1], f32, name="sumexp", tag="sumexp")
                nc.scalar.activation(esc, ps_sc, AF.Exp, bias=negmax,
                                     scale=1.0, accum_out=sumexp)
                recip = small.tile([P, 1], f32, name="recip", tag="recip")
                nc.vector.reciprocal(recip, sumexp)
                if layer == 1:
                    # normalized new-token weight column
                    nc.vector.tensor_scalar_mul(ewall[:, g:g + 1],
                                                esc[:, T:T + 1], recip)

                ps_wt = psmall.tile([P, 2, P], bf16, name="ps_wt", tag="ps_wt")
                for c, cw in tchunks:
                    nc.tensor.transpose(ps_wt[0:cw, c, :],
                                        esc[:, c * P:c * P + cw], ident)
                wt = pwt.tile([P, 2, P], bf16, name="wt", tag="wt")
                for c, cw in tchunks:
                    nc.vector.tensor_copy(wt[0:cw, c, :], ps_wt[0:cw, c, :])

                ps_pt = pspt.tile([P, D], f32, name="ps_pt", tag="ps_pt")
                for kc in range(2):
                    for j in range(4):
                        b = 4 * g + j
                        nc.tensor.matmul(
                            ps_pt[32 * j:32 * j + 32, :],
                            lhsT=wt[:, kc, 32 * j:32 * j + 32],
                            rhs=vts[b][:, kc, :],
                            start=(kc == 0), stop=(kc == 1),
                            tile_position=(0, 32 * j))
                ex = pex.tile([P, D], bf16, name="ex", tag="ex")
                nc.vector.scalar_tensor_tensor(
                    out=ex, in0=ps_pt, scalar=recip, in1=bm4,
                    op0=ALU.mult, op1=ALU.mult)
                ps_a4 = psmall.tile([4, D], f32, name="ps_a4", tag="ps_a4")
                nc.tensor.matmul(ps_a4, lhsT=ones4,
                                 rhs=ex, start=True, stop=True)
                a4 = pa4.tile([4, D], bf16, name="a4", tag="a4")
                nc.vector.tensor_copy(a4, ps_a4)
                if layer == 1:
                    nc.scalar.dma_start(out=dst[4 * g:4 * g + 4, :], in_=a4)
                else:
                    # d-major transpose straight into dst_T (skips the
                    # post-loop batch-major transpose of attn2)
                    ps_tr = psmall.tile([P, 4, 4], bf16, name="ps_tr",
                                        tag="ps_wt")
                    for c in range(4):
                        nc.tensor.transpose(ps_tr[:, c, :],
                                            a4[0:4, c * P:(c + 1) * P],
                                            ident[0:4, 0:4])
                    o = dst_T[:, :, :]
                    dT = bass.AP(tensor=o.tensor, offset=o.offset + 4 * g,
                                 ap=[list(o.ap[0]), [BC, 4], [1, 4]])
                    nc.vector.tensor_copy(dT, ps_tr)

        attention(q_saT, l1_kt, l1_vt, None, attn1, layer=1)

        # band layout [32j+h, g] -> batch-major wxp[4g+j, h] via DRAM bounce
        ewd = dscr.tile([BC, DH], f32, name="ewd", tag="ewd")
        _e = ewd[:, :]
        nc.scalar.dma_start(
            out=bass.AP(tensor=_e.tensor, offset=_e.offset,
                        ap=[[DH, 4], [1, DH], [4 * DH, NG]]),
            in_=ewall)
        nc.scalar.dma_start(out=wxp, in_=ewd[:, 0:H])

        # add the new-token value contribution: attn1[b, d] += wxp[b, head(d)] * v[b, d]
        wxv = big_tmp.tile([BC, D], f32, name="wxv", tag="wxv")
        _w = wxp[:, :]
        wx_b = bass.AP(tensor=_w.tensor, offset=_w.offset,
                       ap=[list(_w.ap[0]), [1, H], [0, DH]])
        nc.vector.tensor_mul(wxv.rearrange("b (h e) -> b h e", h=H), wx_b,
                             v_bm.rearrange("b (h e) -> b h e", h=H))
        nc.vector.tensor_add(attn1, attn1, wxv)

        x1_bm = acts.tile([BC, D], f32, name="x1_bm", tag="x1_bm")
        x1T = acts.tile([P, 4, BC], bf16, name="x1T", tag="x1T")
        attn1T = acts.tile([P, 4, BC], bf16, name="attn1T", tag="attn1T")
        transpose_bm(attn1T, attn1)
        residual_ln(x1_bm, x1T, attn1T, "W0_sa", h_bm, 0)

        # ------------------------------------------------------------------
        # layer 2: encoder attention (masked, padded keys)
        # ------------------------------------------------------------------
        q_aT = acts.tile([P, 4, BC], bf16, name="q_aT", tag="q_aT")
        proj_dmajor(q_aT, "Wq_a", x1T)
        attn2T = acts.tile([P, 4, BC], bf16, name="attn2T", tag="attn2T")
        attention(q_aT, l2_kt, l2_vt, l2_mt, None, layer=2, dst_T=attn2T)

        x2_bm = acts.tile([BC, D], f32, name="x2_bm", tag="x2_bm")
        x2T = acts.tile([P, 4, BC], bf16, name="x2T", tag="x2T")
        residual_ln(x2_bm, x2T, attn2T, "W0_a", x1_bm, 1)

        # ------------------------------------------------------------------
        # MLP
        # ------------------------------------------------------------------
        h1T = acts.tile([P, 4, BC], bf16, name="h1T", tag="h1T")
        proj_dmajor(h1T, "W1", x2T, relu=True)
        x3_bm = acts.tile([BC, D], f32, name="x3_bm", tag="x3_bm")
        x3T = acts.tile([P, 4, BC], bf16, name="x3T", tag="x3T")
        residual_ln(x3_bm, x3T, h1T, "W2", x2_bm, 2)

        qfT = acts.tile([P, 4, BC], bf16, name="qfT", tag="qfT")
        proj_dmajor(qfT, "Wqf", x3T)

        # ------------------------------------------------------------------
        # final pointer scores: w = softmax(10*tanh(qf.K/sqrt(D)) + mask)
        # ------------------------------------------------------------------
        with (
            tc.tile_pool(name="ft", bufs=2) as pft,
            tc.tile_pool(name="fe", bufs=2) as pfe,
            tc.tile_pool(name="fw", bufs=2) as pfw,
        ):
            for g in range(NG):
                # alternate between the two held PSUM pools (pspt is idle in
                # this phase) for a 4-deep group pipeline; no memset: rows
                # not written by the matmuls are never read (tanh bounds
                # them and wf only reads rows 32j)
                pool_f = psc if g % 2 == 0 else pspt
                tag_f = "ps_sc" if g % 2 == 0 else "ps_pt"
                ps_f = pool_f.tile([P, NP], f32, name="ps_f", tag=tag_f)
                for c in range(4):
                    for j in range(4):
                        b = 4 * g + j
                        nc.tensor.matmul(
                            ps_f[32 * j:32 * j + 1, :],
                            lhsT=qfT[:, c, b:b + 1],
                            rhs=fin_kt[b][:, c, :],
                            start=(c == 0), stop=(c == 3),
                            tile_position=(0, 32 * j))
                t1 = pft.tile([P, NP], f32, name="t1", tag="t1")
                nc.scalar.activation(t1, ps_f, AF.Tanh, scale=float(D) ** -0.5)
                t2 = pft.tile([P, NP], f32, name="t2", tag="t2")
                nc.vector.scalar_tensor_tensor(out=t2, in0=t1, scalar=10.0,
                                               in1=fin_mt[g], op0=ALU.mult,
                                               op1=ALU.add)
                e = pfe.tile([P, NP], f32, name="e", tag="e")
                sumexp = small.tile([P, 1], f32, name="fsum", tag="fsum")
                nc.scalar.activation(e, t2, AF.Exp, accum_out=sumexp)
                recip = small.tile([P, 1], f32, name="frec", tag="frec")
                nc.vector.reciprocal(recip, sumexp)
                wf = pfw.tile([P, NK], f32, name="wf", tag="wf")
                nc.vector.tensor_scalar_mul(wf, e[:, 0:NK], recip)
                nc.scalar.dma_start(
                    out=out_d[4 * g:4 * g + 4, :],
                    in_=wf.rearrange("(a b) n -> a b n", b=32)[:, 0, :])

    nc.compile()
    return nc


# ----------------------------------------------------------------------------
# host side
# ----------------------------------------------------------------------------

def _get_program(flags):
    if flags not in _cache:
        _cache[flags] = _build_program(flags)
    return _cache[flags]


def _prep_inputs(inputs):
    """Host-side sharding + layout prep; returns (flags, per-core input maps)."""
    f = np.float32
    bf = ml_dtypes.bfloat16
    f8 = ml_dtypes.float8_e4m3
    h_t = np.asarray(inputs["h_t"], f)
    K_att = np.asarray(inputs["K_att"], f)
    V_att = np.asarray(inputs["V_att"], f)
    K_sa = np.asarray(inputs["K_sa"], f)
    V_sa = np.asarray(inputs["V_sa"], f)
    mask = np.asarray(inputs["mask"])

    sc = np.float32(DH ** -0.5)
    W = {n: np.asarray(inputs[n], f) for n in WNAMES}
    W["Wq_sa"] = W["Wq_sa"] * sc
    W["Wq_a"] = W["Wq_a"] * sc
    Wb = {n: np.ascontiguousarray(W[n]).astype(bf) for n in WNAMES}
    bias_src = {"Wq_sa": "bq_sa", "Wk_sa": "bk_sa", "Wv_sa": "bv_sa",
                "W0_sa": "b0_sa", "Wq_a": "bq_a", "W0_a": "b0_a",
                "W1": "b1", "W2": "b2", "Wqf": "bqf"}
    bvec = {n: np.asarray(inputs[bias_src[n]], f).copy() for n in WNAMES}
    bvec["Wq_sa"] *= sc
    bvec["Wq_a"] *= sc
    use_bias = tuple(bool(np.any(bvec[n])) for n in WNAMES)
    ub = dict(zip(WNAMES, use_bias))

    lnp = np.stack([np.asarray(inputs[k], f) for k in
                    ["ln1_g", "ln1_b", "ln2_g", "ln2_b", "ln3_g", "ln3_b"]])
    ln_affine = tuple(
        bool(np.any(lnp[2 * i] != 1.0) or np.any(lnp[2 * i + 1] != 0.0))
        for i in range(3))
    flags = (use_bias, ln_affine)

    # big transposed streams (full batch, then sliced per core)
    hT = np.ascontiguousarray(
        h_t.reshape(NCORES, BC, D).transpose(0, 2, 1)).astype(bf)
    ksaT = np.ascontiguousarray(
        K_sa.transpose(0, 2, 1).reshape(B, 4, P, T).transpose(0, 2, 1, 3)
    ).astype(bf)                                                  # (B, P, 4, T)
    vsa = np.ascontiguousarray(
        V_sa.reshape(B, 2, P, D).transpose(0, 2, 1, 3)).astype(f8)  # (B, P, 2, D)
    ka0T = np.zeros((B, D, NP), bf)
    ka0T[:, :, :NK] = K_att[:, :, :D].transpose(0, 2, 1).astype(bf)
    ka0T = np.ascontiguousarray(
        ka0T.reshape(B, 4, P, NP).transpose(0, 2, 1, 3))             # (B, P, 4, NP)
    kafT = np.zeros((B, D, NP), bf)
    kafT[:, :, :NK] = K_att[:, :, D:].transpose(0, 2, 1).astype(bf)
    kafT = np.ascontiguousarray(
        kafT.reshape(B, 4, P, NP).transpose(0, 2, 1, 3))             # (B, P, 4, NP)
    va0 = np.zeros((B, NP, D), f8)                                   # keys padded
    va0[:, :NK, :] = V_att[:, :, :D].astype(f8)
    va0 = np.ascontiguousarray(
        va0.reshape(B, 2, P, D).transpose(0, 2, 1, 3))               # (B, P, 2, D)

    # compact mask rows: -1e9 at masked keys and padding, 0 elsewhere
    maskc = np.full((B, NP), -1e9, f)
    maskc[:, :NK] = np.where(mask, f(-1e9), f(0.0))
    maskc = maskc.astype(bf)

    # constant masks
    ident = np.eye(P, dtype=f).astype(bf)
    ones4 = np.zeros((P, 4), f)
    for j in range(4):
        ones4[32 * j:32 * j + 16, j] = 1.0
    ones4 = ones4.astype(bf)
    bm4 = np.zeros((P, D), f)
    for j in range(4):
        for hh in range(H):
            bm4[32 * j + hh, DH * hh:DH * (hh + 1)] = 1.0
    # Qbd mask: for d-chunk c, row p maps to d = 128c + p, head = d // 32
    qm = np.zeros((P, 4 * DH), f)
    for c in range(4):
        for p in range(P):
            hh = (c * P + p) // DH
            qm[p, c * DH + hh] = 1.0
    qm = qm.astype(bf)

    b_dmaj = {n: np.ascontiguousarray(bvec[n].reshape(4, P).T) for n in WNAMES}

    in_maps = []
    for i in range(NCORES):
        sl = slice(BC * i, BC * (i + 1))
        m = {
            "hT": hT[i],
            "h_bm": np.ascontiguousarray(h_t[sl]),
            "ksaT": ksaT[sl],
            "vsa": vsa[sl],
            "ka0T": ka0T[sl],
            "kafT": kafT[sl],
            "va0": va0[sl],
            "maskc": maskc[sl],
            "ident": ident,
            "ones4": ones4,
            "bm4": bm4,
            "qm": qm,
        }
        for n in WNAMES:
            m["W_" + n] = Wb[n]
            if ub[n]:
                m["b_" + n] = b_dmaj[n]
                if n in BM_BIAS:
                    m["bf_" + n] = bvec[n].reshape(1, D)
        if any(ln_affine):
            m["lnp"] = lnp
        in_maps.append(m)
    return flags, in_maps


def _run(inputs, trace=False):
    flags, in_maps = _prep_inputs(inputs)
    nc = _get_program(flags)
    kwargs = {}
    if trace:
        kwargs = dict(trace=True, trace_cores=[0])
    res = run_bass_kernel_spmd(nc, in_maps, list(range(NCORES)), **kwargs)
    out = np.concatenate([res.results[i]["out"] for i in range(NCORES)], axis=0)
    return np.ascontiguousarray(out.astype(np.float32)), res


def kernel(**inputs):
    return _run(inputs, trace=False)[0]


def kernel_traced(**inputs):
    return _run(inputs, trace=True)


# revision 49
# speedup vs baseline: 1.0473x; 1.0473x over previous
"""Trainium2 Bass kernel for nn_AttentionModel (pointer-network decode step).

Data-parallel over 8 NeuronCores: batch 512 -> 64 samples/core; weights
replicated.  Per core the kernel streams the per-sample K/V slabs from HBM
once (~75 MB in bf16/fp8) and computes:

  self-attn over (K_sa | k_sa) -> LN -> enc attention (masked) -> LN ->
  MLP -> LN -> single-head tanh-clipped pointer scores -> softmax weights.

Key structure:
  - K streams (bf16, host-transposed d-major) and V streams (fp8e4m3,
    keys-major) feed per-head score / AV matmuls; softmax, LN, residuals
    and the final output stay fp32.  Mixed bf16-weight x fp8-value
    matmuls are used for AV.
  - ALL stream-load DMA triggers are emitted up front on the sync/gpsimd
    queues (alternating per sample; a small critical prefix of weights
    rides ahead of the floods); ring-slot semaphores pace them, so
    prefetch flows straight through layer transitions instead of stalling
    behind compute program order, and no compute engine ever blocks on a
    starved DMA ring.
  - per-sample 16-head scores live in PSUM as 32-row bands at 32-aligned
    offsets (4 samples per [128, nkeys] tile via PE column-group packing).
    PE transposes of the bf16 weight tile, a block-diagonal head mask and
    a block-diagonal ones-matmul fold per-head value products back to
    [4, 512] batch-major rows.
  - the appended self-attn token's score column is computed once for all
    64 samples (qm-masked q*k reduction) and injected via a DRAM-bounce
    rearrange; its value contribution is a rank-1 elementwise update of
    attn1 (no PE work).
"""

import numpy as np
import ml_dtypes
from contextlib import ExitStack

import concourse.bass as bass
import concourse.tile as tile
from concourse import bacc, mybir
from concourse.bass_utils import run_bass_kernel_spmd

f32 = mybir.dt.float32
bf16 = mybir.dt.bfloat16
fp8 = mybir.dt.float8e4
AF = mybir.ActivationFunctionType
ALU = mybir.AluOpType
AX = mybir.AxisListType

P = 128          # SBUF partitions
NCORES = 8
B = 512          # full batch
BC = B // NCORES # batch per core (64)
D = 512          # model dim
H = 16           # heads
DH = 32          # head dim
NK = 251         # encoder keys (nb_nodes + 1)
NP = 256         # encoder keys padded to 256
T = 256          # self-attn cache length (new token appended on device)
NG = BC // 4     # sample groups of 4 (one [128, n] psum tile each)

WNAMES = ["Wq_sa", "Wk_sa", "Wv_sa", "W0_sa", "Wq_a", "W0_a", "W1", "W2", "Wqf"]
# weight matmuls whose bias is applied on batch-major [64, 512] rows
BM_BIAS = {"Wv_sa", "W0_sa", "W0_a", "W2"}

_cache = {}


# ----------------------------------------------------------------------------
# program builder
# ----------------------------------------------------------------------------

def _build_program(flags):
    """flags = (use_bias tuple aligned with WNAMES, ln_affine tuple of 3)."""
    use_bias = dict(zip(WNAMES, flags[0]))
    ln_affine = flags[1]

    nc = bacc.Bacc("TRN2", target_bir_lowering=False, debug=False)

    def din(name, shape, dt=f32):
        return nc.dram_tensor(name, shape, dt, kind="ExternalInput").ap()

    hT_d = din("hT", [D, BC], bf16)
    hbm_d = din("h_bm", [BC, D])
    ksaT_d = din("ksaT", [BC, P, 4, T], bf16)
    vsa_d = din("vsa", [BC, P, 2, D], fp8)
    ka0T_d = din("ka0T", [BC, P, 4, NP], bf16)
    kafT_d = din("kafT", [BC, P, 4, NP], bf16)
    va0_d = din("va0", [BC, P, 2, D], fp8)
    maskc_d = din("maskc", [BC, NP], bf16)   # compact -1e9/0 mask rows
    W_d = {n: din("W_" + n, [D, D], bf16) for n in WNAMES}
    b_d = {n: din("b_" + n, [P, 4]) for n in WNAMES if use_bias[n]}
    bf_d = {n: din("bf_" + n, [1, D]) for n in WNAMES
            if use_bias[n] and n in BM_BIAS}
    if any(ln_affine):
        lnp_d = din("lnp", [6, D])
    ident_d = din("ident", [P, P], bf16)
    ones4_d = din("ones4", [P, 4], bf16)
    bm4_d = din("bm4", [P, D])
    qm_d = din("qm", [P, 4 * DH], bf16)

    out_d = nc.dram_tensor("out", [BC, NK], f32, kind="ExternalOutput").ap()

    def bcast_row(ap2d, i, n):
        row = ap2d[i:i + 1, :]
        return bass.AP(tensor=row.tensor, offset=row.offset,
                       ap=[[0, n]] + list(row.ap)[1:])

    def mask_bcast(g):
        """[128, NP] tile AP reading maskc rows 4g..4g+4, each row x32."""
        m = maskc_d[4 * g:4 * g + 4, :]
        return bass.AP(tensor=m.tensor, offset=m.offset,
                       ap=[[NP, 4], [0, 32], [1, NP]])

    with tile.TileContext(nc) as tc, ExitStack() as ctx:
        consts = ctx.enter_context(tc.tile_pool(name="consts", bufs=1))
        acts = ctx.enter_context(tc.tile_pool(name="acts", bufs=1))
        small = ctx.enter_context(tc.tile_pool(name="small", bufs=16))
        big_tmp = ctx.enter_context(tc.tile_pool(name="big_tmp", bufs=2))
        dscr = ctx.enter_context(tc.tile_pool(name="dscr", bufs=1, space="DRAM"))
        # streaming pools: one ring each across l1/l2/final
        pk = ctx.enter_context(tc.tile_pool(name="kstream", bufs=42))
        pv = ctx.enter_context(tc.tile_pool(name="vstream", bufs=12))
        pm = ctx.enter_context(tc.tile_pool(name="mp", bufs=16))
        pqbd = ctx.enter_context(tc.tile_pool(name="qbdp", bufs=3))
        pw = ctx.enter_context(tc.tile_pool(name="escp", bufs=3))
        pwt = ctx.enter_context(tc.tile_pool(name="wtp", bufs=3))
        pex = ctx.enter_context(tc.tile_pool(name="exp", bufs=3))
        pa4 = ctx.enter_context(tc.tile_pool(name="a4p", bufs=3))
        # attention PSUM pools (12KB/partition held; 4KB left for proj/residual)
        psc = ctx.enter_context(tc.tile_pool(name="pscp", bufs=2, space="PSUM"))
        pspt = ctx.enter_context(tc.tile_pool(name="psptp", bufs=2, space="PSUM"))
        psmall = ctx.enter_context(tc.tile_pool(name="psmallp", bufs=1, space="PSUM"))

        # ------------------------------------------------------------------
        # constants / weights (sync queue: needed-first weights + h; scalar
        # queue: the rest)
        # ------------------------------------------------------------------
        # all pre-compute loads go on the sync queue (a pure trigger engine)
        # BEFORE the stream floods, so no compute engine ever stalls on a
        # starved DMA ring.
        # critical startup prefix on the sync queue: only what the first
        # attention groups need; everything needed later prefixes the gpsimd
        # queue so neither delays the other or stalls a compute engine.
        x0T = acts.tile([P, 4, BC], bf16, name="x0T", tag="x0T")
        nc.sync.dma_start(out=x0T, in_=hT_d.rearrange("(c p) b -> p c b", p=P))
        Wt, bt, bft = {}, {}, {}
        EARLY_W = ("Wq_sa", "Wk_sa", "Wv_sa")
        for n in WNAMES:
            Wt[n] = consts.tile([P, 4, D], bf16, name="W_" + n, tag="W_" + n)
            if n in EARLY_W:
                nc.sync.dma_start(out=Wt[n],
                                  in_=W_d[n].rearrange("(c p) m -> p c m", p=P))
        qm = consts.tile([P, 4 * DH], bf16, name="qm", tag="qm")
        nc.sync.dma_start(out=qm, in_=qm_d)
        ident = consts.tile([P, P], bf16, name="ident", tag="ident")
        nc.sync.dma_start(out=ident, in_=ident_d)
        ones4 = consts.tile([P, 4], bf16, name="ones4", tag="ones4")
        nc.sync.dma_start(out=ones4, in_=ones4_d)
        bm4 = consts.tile([P, D], f32, name="bm4", tag="bm4")
        nc.sync.dma_start(out=bm4, in_=bm4_d)
        eps = consts.tile([P, 1], f32, name="eps", tag="eps")
        nc.vector.memset(eps, 1e-5)
        # later-needed loads: gpsimd-queue prefix
        for n in WNAMES:
            if n not in EARLY_W:
                nc.gpsimd.dma_start(out=Wt[n],
                                    in_=W_d[n].rearrange("(c p) m -> p c m", p=P))
        h_bm = acts.tile([BC, D], f32, name="h_bm", tag="h_bm")
        nc.gpsimd.dma_start(out=h_bm, in_=hbm_d)
        for n in WNAMES:
            if use_bias[n]:
                bt[n] = consts.tile([P, 4], f32, name="b_" + n, tag="b_" + n)
                nc.gpsimd.dma_start(out=bt[n], in_=b_d[n])
                if n in BM_BIAS:
                    bft[n] = consts.tile([BC, D], f32, name="bf_" + n, tag="bf_" + n)
                    nc.gpsimd.dma_start(out=bft[n], in_=bcast_row(bf_d[n], 0, BC))

        lng, lnb = [None] * 3, [None] * 3
        for i in range(3):
            if ln_affine[i]:
                lng[i] = consts.tile([BC, D], f32, name=f"lng{i}", tag=f"lng{i}")
                nc.gpsimd.dma_start(out=lng[i], in_=bcast_row(lnp_d, 2 * i, BC))
                lnb[i] = consts.tile([BC, D], f32, name=f"lnb{i}", tag=f"lnb{i}")
                nc.gpsimd.dma_start(out=lnb[i], in_=bcast_row(lnp_d, 2 * i + 1, BC))

        # ------------------------------------------------------------------
        # ALL stream loads, emitted up front (ring semaphores pace them),
        # alternating queues per sample for balanced bytes
        # ------------------------------------------------------------------
        l1_kt, l1_vt, l2_kt, l2_vt, fin_kt = [], [], [], [], []
        l2_mt, fin_mt = [], []
        for b in range(BC):
            ek = nc.sync if b % 2 == 0 else nc.gpsimd
            ev = nc.gpsimd if b % 2 == 0 else nc.sync
            kt = pk.tile([P, 4, T], bf16, name="kt4", tag="kt4")
            ek.dma_start(out=kt, in_=ksaT_d[b])
            l1_kt.append(kt)
            vt = pv.tile([P, 2, D], fp8, name="vt2", tag="vt2")
            ev.dma_start(out=vt, in_=vsa_d[b])
            l1_vt.append(vt)
        for b in range(BC):
            ek = nc.sync if b % 2 == 0 else nc.gpsimd
            ev = nc.gpsimd if b % 2 == 0 else nc.sync
            kt = pk.tile([P, 4, NP], bf16, name="kt4", tag="kt4")
            ek.dma_start(out=kt, in_=ka0T_d[b])
            l2_kt.append(kt)
            vt = pv.tile([P, 2, D], fp8, name="vt2", tag="vt2")
            ev.dma_start(out=vt, in_=va0_d[b])
            l2_vt.append(vt)
        for b in range(BC):
            kt = pk.tile([P, 4, NP], bf16, name="kt4", tag="kt4")
            eng = nc.sync if b % 2 == 0 else nc.gpsimd
            eng.dma_start(out=kt, in_=kafT_d[b])
            fin_kt.append(kt)
        for g in range(NG):
            mt = pm.tile([P, NP], bf16, name="mt2", tag="mt2")
            nc.gpsimd.dma_start(out=mt, in_=mask_bcast(g))
            l2_mt.append(mt)
        fin_mt = l2_mt

        # ------------------------------------------------------------------
        # helpers
        # ------------------------------------------------------------------
        def proj_dmajor(dst, wname, src_T, relu=False):
            """dst[:, mc, :] (d-major bf16 [128, 4, 64]) = act(x @ W + b)."""
            with tc.tile_pool(name="pp_" + wname, bufs=2, space="PSUM") as pp:
                for mc in range(4):
                    ps = pp.tile([P, BC], f32, name="ps", tag="ps")
                    for kc in range(4):
                        nc.tensor.matmul(
                            ps,
                            lhsT=Wt[wname][:, kc, mc * P:(mc + 1) * P],
                            rhs=src_T[:, kc, :],
                            start=(kc == 0), stop=(kc == 3),
                        )
                    bias = bt[wname][:, mc:mc + 1] if use_bias[wname] else 0.0
                    func = AF.Relu if relu else AF.Identity
                    nc.scalar.activation(dst[:, mc, :], ps, func, bias=bias, scale=1.0)

        def mm_batchmajor(ps, src_T, wname):
            """ps [64, 512] = x @ W   (lhsT = x^T chunks, W as moving)."""
            for kc in range(4):
                nc.tensor.matmul(
                    ps,
                    lhsT=src_T[:, kc, :],
                    rhs=Wt[wname][:, kc, :],
                    start=(kc == 0), stop=(kc == 3),
                )

        def transpose_bm(dst_T, src_bm):
            """[64, 512] bf16 batch-major -> d-major bf16 [128, 4, 64]."""
            with tc.tile_pool(name="ptr", bufs=2, space="PSUM") as pt:
                for c in range(4):
                    ps = pt.tile([P, BC], bf16, name="ps", tag="ps")
                    nc.tensor.transpose(ps, src_bm[:, c * P:(c + 1) * P],
                                        ident[0:BC, 0:BC])
                    nc.scalar.copy(dst_T[:, c, :], ps)

        def layer_norm(dst_bm, t_bm, idx):
            stats = small.tile([BC, 6], f32, name="stats", tag="stats")
            nc.vector.bn_stats(stats, t_bm)
            mv = small.tile([BC, 2], f32, name="mv", tag="mv")
            nc.vector.bn_aggr(mv, stats)
            sd = small.tile([BC, 1], f32, name="sd", tag="sd")
            nc.scalar.activation(sd, mv[:, 1:2], AF.Sqrt, bias=eps[0:BC], scale=1.0)
            rstd = small.tile([BC, 1], f32, name="rstd", tag="rstd")
            nc.vector.reciprocal(rstd, sd)
            nmr = small.tile([BC, 1], f32, name="nmr", tag="nmr")
            nc.vector.scalar_tensor_tensor(out=nmr, in0=mv[:, 0:1], scalar=-1.0,
                                           in1=rstd, op0=ALU.mult, op1=ALU.mult)
            if ln_affine[idx]:
                xn = big_tmp.tile([BC, D], f32, name="xn", tag="xn")
                nc.scalar.activation(xn, t_bm, AF.Identity, bias=nmr, scale=rstd)
                nc.vector.tensor_mul(xn, xn, lng[idx])
                nc.vector.tensor_add(dst_bm, xn, lnb[idx])
            else:
                nc.scalar.activation(dst_bm, t_bm, AF.Identity, bias=nmr, scale=rstd)

        def residual_ln(dst_bm, dst_T, src_T, wname, x_prev_bm, idx):
            """dst = LN(x_prev + src @ W + b); also d-major bf16 dst_T."""
            with tc.tile_pool(name="pr_" + wname, bufs=1, space="PSUM") as pr:
                ps = pr.tile([BC, D], f32, name="ps", tag="ps")
                mm_batchmajor(ps, src_T, wname)
                t_bm = big_tmp.tile([BC, D], f32, name="t_bm", tag="t_bm")
                nc.vector.tensor_add(t_bm, ps, x_prev_bm)
                if use_bias[wname]:
                    nc.vector.tensor_add(t_bm, t_bm, bft[wname])
                layer_norm(dst_bm, t_bm, idx)
            dst_bf = big_tmp.tile([BC, D], bf16, name="dbf", tag="dbf_" + wname)
            nc.scalar.copy(dst_bf, dst_bm)
            transpose_bm(dst_T, dst_bf)

        def qbd_group(dst, src_T, g):
            """dst [P, 16, DH] bf16: block-diag per-head queries, 4 samples.

            dst[:, 4j+c, :] = qm[:, c*DH:(c+1)*DH] * src_T[:, c, 4g+j]
            done in one broadcast vector op (free dims j, c, dh).
            """
            o = dst[:, :, :]
            a = src_T[:, :, :]
            m = qm[:, :]
            out_ap = bass.AP(tensor=o.tensor, offset=o.offset,
                             ap=[list(o.ap[0]), [4 * DH, 4], [DH, 4], [1, DH]])
            in1 = bass.AP(tensor=a.tensor, offset=a.offset + 4 * g,
                          ap=[list(a.ap[0]), [1, 4], [BC, 4], [0, DH]])
            in0 = bass.AP(tensor=m.tensor, offset=m.offset,
                          ap=[list(m.ap[0]), [0, 4], [DH, 4], [1, DH]])
            nc.vector.tensor_mul(out_ap, in0, in1)

        # ------------------------------------------------------------------
        # projections from x0 = h_t
        # ------------------------------------------------------------------
        q_saT = acts.tile([P, 4, BC], bf16, name="q_saT", tag="q_saT")
        proj_dmajor(q_saT, "Wq_sa", x0T)
        k_saT = acts.tile([P, 4, BC], bf16, name="k_saT", tag="k_saT")
        proj_dmajor(k_saT, "Wk_sa", x0T)

        v_bm = acts.tile([BC, D], bf16, name="v_bm", tag="v_bm")
        with tc.tile_pool(name="pv0", bufs=1, space="PSUM") as pv0:
            psv = pv0.tile([BC, D], f32, name="psv", tag="psv")
            mm_batchmajor(psv, x0T, "Wv_sa")
            if use_bias["Wv_sa"]:
                nc.vector.tensor_add(v_bm, psv, bft["Wv_sa"])
            else:
                nc.scalar.copy(v_bm, psv)

        # extra-token self-attn scores for all samples, batch-major:
        # sxb[b, h] = q_b . k_b per head (cols h>=16 exact 0), bounced
        # through DRAM into band layout sx4[32j+h, g].
        qk = acts.tile([P, 4, BC], bf16, name="qk", tag="qk")
        nc.vector.tensor_mul(qk, q_saT, k_saT)
        sxb = acts.tile([BC, DH], f32, name="sxb", tag="sxb")
        with tc.tile_pool(name="px", bufs=1, space="PSUM") as px:
            ps_x = px.tile([BC, DH], f32, name="ps_x", tag="ps_x")
            for c in range(4):
                nc.tensor.matmul(ps_x, lhsT=qk[:, c, :],
                                 rhs=qm[:, c * DH:(c + 1) * DH],
                                 start=(c == 0), stop=(c == 3))
            nc.scalar.copy(sxb, ps_x)
        sxd = dscr.tile([BC, DH], f32, name="sxd", tag="sxd")
        nc.scalar.dma_start(out=sxd, in_=sxb)
        sx4 = acts.tile([P, NG], f32, name="sx4", tag="sx4")
        _s = sxd[:, :]
        nc.scalar.dma_start(
            out=sx4,
            in_=bass.AP(tensor=_s.tensor, offset=_s.offset,
                        ap=[[DH, 4], [1, DH], [4 * DH, NG]]))

        # ------------------------------------------------------------------
        # layer 1: self-attention over (K_sa | k_sa)
        # ------------------------------------------------------------------
        attn1 = acts.tile([BC, D], bf16, name="attn1", tag="attn1")
        ewall = acts.tile([P, NG], f32, name="ewall", tag="ewall")
        wxp = acts.tile([BC, H], f32, name="wxp", tag="wxp")

        def attention(q_T, kts, vts, mts, dst, layer, dst_T=None):
            wcols = T + 1 if layer == 1 else NP
            tchunks = [(0, P), (1, P)]
            for g in range(NG):
                qbd = pqbd.tile([P, 16, DH], bf16, name="qbd", tag="qbd")
                qbd_group(qbd, q_T, g)
                ps_sc = psc.tile([P, wcols], f32, name="ps_sc", tag="ps_sc")
                for c in range(4):
                    for j in range(4):
                        b = 4 * g + j
                        nc.tensor.matmul(
                            ps_sc[32 * j:32 * j + 32, 0:NP],
                            lhsT=qbd[:, 4 * j + c, :],
                            rhs=kts[b][:, c, :],
                            start=(c == 0), stop=(c == 3),
                            tile_position=(0, 32 * j))
                if layer == 1:
                    nc.scalar.copy(ps_sc[:, T:T + 1], sx4[:, g:g + 1])
                else:
                    nc.vector.tensor_add(ps_sc, ps_sc, mts[g])
                negmax = small.tile([P, 1], f32, name="negmax", tag="negmax")
                nc.vector.tensor_reduce(negmax, ps_sc, axis=AX.X,
                                        op=ALU.max, negate=True)
                esc = pw.tile([P, wcols], bf16, name="esc", tag="esc")
                sumexp = small.tile([P, 1], f32, name="sumexp", tag="sumexp")
                nc.scalar.activation(esc, ps_sc, AF.Exp, bias=negmax,
                                     scale=1.0, accum_out=sumexp)
                recip = small.tile([P, 1], f32, name="recip", tag="recip")
                nc.vector.reciprocal(recip, sumexp)
                if layer == 1:
                    # normalized new-token weight column
                    nc.vector.tensor_scalar_mul(ewall[:, g:g + 1],
                                                esc[:, T:T + 1], recip)

                ps_wt = psmall.tile([P, 2, P], bf16, name="ps_wt", tag="ps_wt")
                for c, cw in tchunks:
                    nc.tensor.transpose(ps_wt[0:cw, c, :],
                                        esc[:, c * P:c * P + cw], ident)
                wt = pwt.tile([P, 2, P], bf16, name="wt", tag="wt")
                for c, cw in tchunks:
                    nc.vector.tensor_copy(wt[0:cw, c, :], ps_wt[0:cw, c, :])

                ps_pt = pspt.tile([P, D], f32, name="ps_pt", tag="ps_pt")
                for kc in range(2):
                    for j in range(4):
                        b = 4 * g + j
                        nc.tensor.matmul(
                            ps_pt[32 * j:32 * j + 32, :],
                            lhsT=wt[:, kc, 32 * j:32 * j + 32],
                            rhs=vts[b][:, kc, :],
                            start=(kc == 0), stop=(kc == 1),
                            tile_position=(0, 32 * j))
                ex = pex.tile([P, D], bf16, name="ex", tag="ex")
                nc.vector.scalar_tensor_tensor(
                    out=ex, in0=ps_pt, scalar=recip, in1=bm4,
                    op0=ALU.mult, op1=ALU.mult)
                ps_a4 = psmall.tile([4, D], f32, name="ps_a4", tag="ps_a4")
                nc.tensor.matmul(ps_a4, lhsT=ones4,
                                 rhs=ex, start=True, stop=True)
                a4 = pa4.tile([4, D], bf16, name="a4", tag="a4")
                nc.vector.tensor_copy(a4, ps_a4)
                if layer == 1:
                    nc.scalar.dma_start(out=dst[4 * g:4 * g + 4, :], in_=a4)
                else:
                    # d-major transpose straight into dst_T (skips the
                    # post-loop batch-major transpose of attn2)
                    ps_tr = psmall.tile([P, 4, 4], bf16, name="ps_tr",
                                        tag="ps_wt")
                    for c in range(4):
                        nc.tensor.transpose(ps_tr[:, c, :],
                                            a4[0:4, c * P:(c + 1) * P],
                                            ident[0:4, 0:4])
                    o = dst_T[:, :, :]
                    dT = bass.AP(tensor=o.tensor, offset=o.offset + 4 * g,
                                 ap=[list(o.ap[0]), [BC, 4], [1, 4]])
                    nc.vector.tensor_copy(dT, ps_tr)

        attention(q_saT, l1_kt, l1_vt, None, attn1, layer=1)

        # band layout [32j+h, g] -> batch-major wxp[4g+j, h] via DRAM bounce
        ewd = dscr.tile([BC, DH], f32, name="ewd", tag="ewd")
        _e = ewd[:, :]
        nc.scalar.dma_start(
            out=bass.AP(tensor=_e.tensor, offset=_e.offset,
                        ap=[[DH, 4], [1, DH], [4 * DH, NG]]),
            in_=ewall)
        nc.scalar.dma_start(out=wxp, in_=ewd[:, 0:H])

        # add the new-token value contribution: attn1[b, d] += wxp[b, head(d)] * v[b, d]
        wxv = big_tmp.tile([BC, D], f32, name="wxv", tag="wxv")
        _w = wxp[:, :]
        wx_b = bass.AP(tensor=_w.tensor, offset=_w.offset,
                       ap=[list(_w.ap[0]), [1, H], [0, DH]])
        nc.vector.tensor_mul(wxv.rearrange("b (h e) -> b h e", h=H), wx_b,
                             v_bm.rearrange("b (h e) -> b h e", h=H))
        nc.vector.tensor_add(attn1, attn1, wxv)

        x1_bm = acts.tile([BC, D], f32, name="x1_bm", tag="x1_bm")
        x1T = acts.tile([P, 4, BC], bf16, name="x1T", tag="x1T")
        attn1T = acts.tile([P, 4, BC], bf16, name="attn1T", tag="attn1T")
        transpose_bm(attn1T, attn1)
        residual_ln(x1_bm, x1T, attn1T, "W0_sa", h_bm, 0)

        # ------------------------------------------------------------------
        # layer 2: encoder attention (masked, padded keys)
        # ------------------------------------------------------------------
        q_aT = acts.tile([P, 4, BC], bf16, name="q_aT", tag="q_aT")
        proj_dmajor(q_aT, "Wq_a", x1T)
        attn2T = acts.tile([P, 4, BC], bf16, name="attn2T", tag="attn2T")
        attention(q_aT, l2_kt, l2_vt, l2_mt, None, layer=2, dst_T=attn2T)

        x2_bm = acts.tile([BC, D], f32, name="x2_bm", tag="x2_bm")
        x2T = acts.tile([P, 4, BC], bf16, name="x2T", tag="x2T")
        residual_ln(x2_bm, x2T, attn2T, "W0_a", x1_bm, 1)

        # ------------------------------------------------------------------
        # MLP
        # ------------------------------------------------------------------
        h1T = acts.tile([P, 4, BC], bf16, name="h1T", tag="h1T")
        proj_dmajor(h1T, "W1", x2T, relu=True)
        x3_bm = acts.tile([BC, D], f32, name="x3_bm", tag="x3_bm")
        x3T = acts.tile([P, 4, BC], bf16, name="x3T", tag="x3T")
        residual_ln(x3_bm, x3T, h1T, "W2", x2_bm, 2)

        qfT = acts.tile([P, 4, BC], bf16, name="qfT", tag="qfT")
        proj_dmajor(qfT, "Wqf", x3T)

        # ------------------------------------------------------------------
        # final pointer scores: w = softmax(10*tanh(qf.K/sqrt(D)) + mask)
        # ------------------------------------------------------------------
        with (
            tc.tile_pool(name="ft", bufs=2) as pft,
            tc.tile_pool(name="fe", bufs=2) as pfe,
            tc.tile_pool(name="fw", bufs=2) as pfw,
        ):
            for g in range(NG):
                # alternate between the two held PSUM pools (pspt is idle in
                # this phase) for a 4-deep group pipeline; no memset: rows
                # not written by the matmuls are never read (tanh bounds
                # them and wf only reads rows 32j)
                pool_f = psc if g % 2 == 0 else pspt
                tag_f = "ps_sc" if g % 2 == 0 else "ps_pt"
                ps_f = pool_f.tile([P, NP], f32, name="ps_f", tag=tag_f)
                for c in range(4):
                    for j in range(4):
                        b = 4 * g + j
                        nc.tensor.matmul(
                            ps_f[32 * j:32 * j + 1, :],
                            lhsT=qfT[:, c, b:b + 1],
                            rhs=fin_kt[b][:, c, :],
                            start=(c == 0), stop=(c == 3),
                            tile_position=(0, 32 * j))
                t1 = pft.tile([P, NP], f32, name="t1", tag="t1")
                nc.scalar.activation(t1, ps_f, AF.Tanh, scale=float(D) ** -0.5)
                t2 = pft.tile([P, NP], f32, name="t2", tag="t2")
                nc.vector.scalar_tensor_tensor(out=t2, in0=t1, scalar=10.0,
                                               in1=fin_mt[g], op0=ALU.mult,
                                               op1=ALU.add)
                e = pfe.tile([P, NP], f32, name="e", tag="e")
                sumexp = small.tile([P, 1], f32, name="fsum", tag="fsum")
                nc.scalar.activation(e, t2, AF.Exp, accum_out=sumexp)
                recip = small.tile([P, 1], f32, name="frec", tag="frec")
                nc.vector.reciprocal(recip, sumexp)
                wf = pfw.tile([P, NK], f32, name="wf", tag="wf")
                nc.vector.tensor_scalar_mul(wf, e[:, 0:NK], recip)
                nc.scalar.dma_start(
                    out=out_d[4 * g:4 * g + 4, :],
                    in_=wf.rearrange("(a b) n -> a b n", b=32)[:, 0, :])

    nc.compile()
    return nc


# ----------------------------------------------------------------------------
# host side
# ----------------------------------------------------------------------------

def _get_program(flags):
    if flags not in _cache:
        _cache[flags] = _build_program(flags)
    return _cache[flags]


def _prep_inputs(inputs):
    """Host-side sharding + layout prep; returns (flags, per-core input maps)."""
    f = np.float32
    bf = ml_dtypes.bfloat16
    f8 = ml_dtypes.float8_e4m3
    h_t = np.asarray(inputs["h_t"], f)
    K_att = np.asarray(inputs["K_att"], f)
    V_att = np.asarray(inputs["V_att"], f)
    K_sa = np.asarray(inputs["K_sa"], f)
    V_sa = np.asarray(inputs["V_sa"], f)
    mask = np.asarray(inputs["mask"])

    sc = np.float32(DH ** -0.5)
    W = {n: np.asarray(inputs[n], f) for n in WNAMES}
    W["Wq_sa"] = W["Wq_sa"] * sc
    W["Wq_a"] = W["Wq_a"] * sc
    Wb = {n: np.ascontiguousarray(W[n]).astype(bf) for n in WNAMES}
    bias_src = {"Wq_sa": "bq_sa", "Wk_sa": "bk_sa", "Wv_sa": "bv_sa",
                "W0_sa": "b0_sa", "Wq_a": "bq_a", "W0_a": "b0_a",
                "W1": "b1", "W2": "b2", "Wqf": "bqf"}
    bvec = {n: np.asarray(inputs[bias_src[n]], f).copy() for n in WNAMES}
    bvec["Wq_sa"] *= sc
    bvec["Wq_a"] *= sc
    use_bias = tuple(bool(np.any(bvec[n])) for n in WNAMES)
    ub = dict(zip(WNAMES, use_bias))

    lnp = np.stack([np.asarray(inputs[k], f) for k in
                    ["ln1_g", "ln1_b", "ln2_g", "ln2_b", "ln3_g", "ln3_b"]])
    ln_affine = tuple(
        bool(np.any(lnp[2 * i] != 1.0) or np.any(lnp[2 * i + 1] != 0.0))
        for i in range(3))
    flags = (use_bias, ln_affine)

    # big transposed streams (full batch, then sliced per core)
    hT = np.ascontiguousarray(
        h_t.reshape(NCORES, BC, D).transpose(0, 2, 1)).astype(bf)
    ksaT = np.ascontiguousarray(
        K_sa.transpose(0, 2, 1).reshape(B, 4, P, T).transpose(0, 2, 1, 3)
    ).astype(bf)                                                  # (B, P, 4, T)
    vsa = np.ascontiguousarray(
        V_sa.reshape(B, 2, P, D).transpose(0, 2, 1, 3)).astype(f8)  # (B, P, 2, D)
    ka0T = np.zeros((B, D, NP), bf)
    ka0T[:, :, :NK] = K_att[:, :, :D].transpose(0, 2, 1).astype(bf)
    ka0T = np.ascontiguousarray(
        ka0T.reshape(B, 4, P, NP).transpose(0, 2, 1, 3))             # (B, P, 4, NP)
    kafT = np.zeros((B, D, NP), bf)
    kafT[:, :, :NK] = K_att[:, :, D:].transpose(0, 2, 1).astype(bf)
    kafT = np.ascontiguousarray(
        kafT.reshape(B, 4, P, NP).transpose(0, 2, 1, 3))             # (B, P, 4, NP)
    va0 = np.zeros((B, NP, D), f8)                                   # keys padded
    va0[:, :NK, :] = V_att[:, :, :D].astype(f8)
    va0 = np.ascontiguousarray(
        va0.reshape(B, 2, P, D).transpose(0, 2, 1, 3))               # (B, P, 2, D)

    # compact mask rows: -1e9 at masked keys and padding, 0 elsewhere
    maskc = np.full((B, NP), -1e9, f)
    maskc[:, :NK] = np.where(mask, f(-1e9), f(0.0))
    maskc = maskc.astype(bf)

    # constant masks
    ident = np.eye(P, dtype=f).astype(bf)
    ones4 = np.zeros((P, 4), f)
    for j in range(4):
        ones4[32 * j:32 * j + 16, j] = 1.0
    ones4 = ones4.astype(bf)
    bm4 = np.zeros((P, D), f)
    for j in range(4):
        for hh in range(H):
            bm4[32 * j + hh, DH * hh:DH * (hh + 1)] = 1.0
    # Qbd mask: for d-chunk c, row p maps to d = 128c + p, head = d // 32
    qm = np.zeros((P, 4 * DH), f)
    for c in range(4):
        for p in range(P):
            hh = (c * P + p) // DH
            qm[p, c * DH + hh] = 1.0
    qm = qm.astype(bf)

    b_dmaj = {n: np.ascontiguousarray(bvec[n].reshape(4, P).T) for n in WNAMES}

    in_maps = []
    for i in range(NCORES):
        sl = slice(BC * i, BC * (i + 1))
        m = {
            "hT": hT[i],
            "h_bm": np.ascontiguousarray(h_t[sl]),
            "ksaT": ksaT[sl],
            "vsa": vsa[sl],
            "ka0T": ka0T[sl],
            "kafT": kafT[sl],
            "va0": va0[sl],
            "maskc": maskc[sl],
            "ident": ident,
            "ones4": ones4,
            "bm4": bm4,
            "qm": qm,
        }
        for n in WNAMES:
            m["W_" + n] = Wb[n]
            if ub[n]:
                m["b_" + n] = b_dmaj[n]
                if n in BM_BIAS:
                    m["bf_" + n] = bvec[n].reshape(1, D)
        if any(ln_affine):
            m["lnp"] = lnp
        in_maps.append(m)
    return flags, in_maps


def _run(inputs, trace=False):
    flags, in_maps = _prep_inputs(inputs)
    nc = _get_program(flags)
    kwargs = {}
    if trace:
        kwargs = dict(trace=True, trace_cores=[0])
    res = run_bass_kernel_spmd(nc, in_maps, list(range(NCORES)), **kwargs)
    out = np.concatenate([res.results[i]["out"] for i in range(NCORES)], axis=0)
    return np.ascontiguousarray(out.astype(np.float32)), res


def kernel(**inputs):
    return _run(inputs, trace=False)[0]


def kernel_traced(**inputs):
    return _run(inputs, trace=True)
